# revision 19
# baseline (speedup 1.0000x reference)
"""Trainium2 Bass kernel for a 6-layer GPT (D=512, H=8, T=1024, B=2, V=50257).

Strategy (8 NeuronCores), v3:
- Token-shard the transformer body: core c owns 256 tokens (cores 0-3 =
  batch 0 chunks 0-3, cores 4-7 = batch 1 chunks 0-3).
- All matmul operands bf16 (PSUM accumulates fp32); residual/LN fp32.
- Host pre-rearranges every weight into partition-major layout so DMA
  descriptors are >=4KB contiguous lines; DMAs are spread over the three
  trigger queues (sync / scalar / gpsimd).
- Per layer: LN1 -> QKV -> AllGather K,V (bf16) across the batch group
  (next layer's weights prefetched at layer start) -> causal attention,
  per-head PSUM accumulation, masks multiplied on the vector engine,
  per-head pipelined softmax normalization -> Wo + residual -> LN2 ->
  MLP -> residual.
- Final LN -> AllGather hidden (8-core, Shared/RDH) overlapped with
  discarded warmup matmuls on the local tokens -> vocab-sharded LM head
  with Wlm resident in SBUF (loaded in 4 slices during layers 0-2);
  logits accumulate 13 vocab tiles in SBUF and go out as one 13KB-per-
  partition DMA per 128-token group, alternating queues.
- Host folds LN gamma/beta and the 1/sqrt(HS) score scale into the
  weights; embedding gather happens host-side (tiny).
"""

import numpy as np
import ml_dtypes

import concourse.bass as bass
import concourse.tile as tile
from concourse import bacc, mybir
from concourse import bass_utils
from concourse.bass import ds, ts
from concourse.masks import make_identity

FP = mybir.dt.float32
BF = mybir.dt.bfloat16
AF = mybir.ActivationFunctionType
OP = mybir.AluOpType

V, D, T, L, H, HS, B = 50257, 512, 1024, 6, 8, 64, 2
FF = 4 * D
EPS = 1e-5
NC = 8          # cores
CH = 256        # tokens per core
VS = 6284       # padded vocab shard per core; 8*VS = 50272 >= V
KD = D // 128   # 4 k-tiles over D
MD = FF // 128  # 16 m-tiles over FF
BT = B * T
NT = (VS + 511) // 512   # 13 vocab tiles per core
NTP = NT * 512           # padded vocab shard (6656)


def build_program(with_bias=True, layers=L):
    nc = bacc.Bacc("TRN2", target_bir_lowering=False, debug=False, num_devices=NC)

    # ---- I/O (weights host-side pre-rearranged to partition-major) ----
    x0 = nc.dram_tensor("x0", [128, 2, D], FP, kind="ExternalInput").ap()
    wq = nc.dram_tensor("wq", [L, 128, KD, D], BF, kind="ExternalInput").ap()
    wk = nc.dram_tensor("wk", [L, 128, KD, D], BF, kind="ExternalInput").ap()
    wv = nc.dram_tensor("wv", [L, 128, KD, D], BF, kind="ExternalInput").ap()
    wo = nc.dram_tensor("wo", [L, 128, KD, D], BF, kind="ExternalInput").ap()
    w1 = nc.dram_tensor("w1", [L, 128, KD, FF], BF, kind="ExternalInput").ap()
    w2 = nc.dram_tensor("w2", [L, 128, MD, D], BF, kind="ExternalInput").ap()
    wlm = nc.dram_tensor("wlm", [128, KD, VS], BF, kind="ExternalInput").ap()
    bqk = nc.dram_tensor("bqk", [128, L, 2, KD], FP, kind="ExternalInput").ap()
    b1t = nc.dram_tensor("b1t", [128, L, MD], FP, kind="ExternalInput").ap()
    bo2 = nc.dram_tensor("bo2", [L, 2, D], FP, kind="ExternalInput").ap()
    # causal 0/1 mask per core: [p, kchunk, ktile, 256 queries] bf16
    msk = nc.dram_tensor("msk", [128, 4, 2, CH], BF, kind="ExternalInput").ap()
    # logits row (tg, p) = token tg*128+p of the gathered order
    logits = nc.dram_tensor("logits", [BT // 128, 128, NTP], BF,
                            kind="ExternalOutput").ap()

    KV_K = 128 * KD * CH            # kT flat elements per core
    KV_V = 128 * 2 * H * 65         # v_aug flat elements per core
    KV = KV_K + KV_V
    XF = 128 * KD * CH              # xfT flat elements

    from contextlib import ExitStack
    with ExitStack() as stk:
        tc = stk.enter_context(tile.TileContext(nc))
        ec = stk.enter_context
        consts = ec(tc.tile_pool(name="consts", bufs=1))
        wlmp = ec(tc.tile_pool(name="wlmp", bufs=1))
        xpool = ec(tc.tile_pool(name="xpool", bufs=1))
        hpool = ec(tc.tile_pool(name="hpool", bufs=1))
        t4 = ec(tc.tile_pool(name="t4", bufs=4))          # [128,KD,CH] transposed acts
        wqkvop = ec(tc.tile_pool(name="wqkvo", bufs=2))   # [128,KD,4,512]
        w1pool = ec(tc.tile_pool(name="w1k", bufs=3))     # [128,KD,FF//2] halves
        w2pool = ec(tc.tile_pool(name="w2k", bufs=3))     # [128,MD//2,D] halves
        kvall = ec(tc.tile_pool(name="kvall", bufs=1))
        vaugp = ec(tc.tile_pool(name="vaug", bufs=1))
        small = ec(tc.tile_pool(name="small", bufs=2))
        expp = ec(tc.tile_pool(name="exp", bufs=2))
        gtp = ec(tc.tile_pool(name="gt", bufs=2))
        attp = ec(tc.tile_pool(name="attp", bufs=1))
        rbcp = ec(tc.tile_pool(name="rbc", bufs=2))
        lgp = ec(tc.tile_pool(name="lg", bufs=2))
        bcp = ec(tc.tile_pool(name="bcast", bufs=2))
        xftp = ec(tc.tile_pool(name="xft", bufs=1))
        xfap = ec(tc.tile_pool(name="xfa", bufs=2))
        mmp = ec(tc.tile_pool(name="mm", bufs=2, space="PSUM"))
        avp = ec(tc.tile_pool(name="avp", bufs=2, space="PSUM"))
        spp = ec(tc.tile_pool(name="sp", bufs=2, space="PSUM"))
        mop = ec(tc.tile_pool(name="mo", bufs=2, space="PSUM"))
        dram = ec(tc.tile_pool(name="dram", bufs=2, space="DRAM"))

        # ---- consts + first activations on the sync queue ----
        xt = xpool.tile([128, 2, D], FP, tag="xt")
        nc.sync.dma_start(xt[:], x0)
        ident = consts.tile([128, 128], BF)
        make_identity(nc, ident)
        epst = consts.tile([128, 1], FP)
        nc.vector.memset(epst, EPS)
        msk_sb = consts.tile([128, 4, 2, CH], BF)
        nc.sync.dma_start(msk_sb[:], msk)
        bqk_sb = consts.tile([128, L, 2, KD], FP)
        b1_sb = consts.tile([128, L, MD], FP)
        if with_bias:
            nc.sync.dma_start(bqk_sb[:], bqk)
            nc.sync.dma_start(b1_sb[:], b1t)

        # ---- layer-0 weights (scalar + gpsimd queues), then resident Wlm --
        def load_wqkvo(l):
            w = wqkvop.tile([128, KD, 4, 512], BF, tag="wqkvo", name=f"wqkvo{l}")
            for i, src in enumerate((wq, wk, wv, wo)):
                eng = nc.scalar if i < 2 else nc.gpsimd
                eng.dma_start(w[:, :, i, :], src[l])
            return w

        def load_w1h(l, i):
            w = w1pool.tile([128, KD, FF // 2], BF, tag="w1k",
                            name=f"w1k{l}_{i}")
            nc.scalar.dma_start(w[:], w1[l, :, :, ds(i * (FF // 2), FF // 2)])
            return w

        def load_w2h(l, i):
            w = w2pool.tile([128, MD // 2, D], BF, tag="w2k",
                            name=f"w2k{l}_{i}")
            nc.gpsimd.dma_start(w[:], w2[l, :, ds(i * (MD // 2), MD // 2), :])
            return w

        cur_wqkvo = load_wqkvo(0)
        cur_w1 = [load_w1h(0, 0), load_w1h(0, 1)]
        cur_w2 = [load_w2h(0, 0), load_w2h(0, 1)]
        wlm_sb = wlmp.tile([128, KD, VS], BF)
        nc.gpsimd.dma_start(wlm_sb[:, 0, :], wlm[:, 0, :])

        def layernorm(src):
            """src [128,2,D] fp32 -> normalized [128,2,D] bf16."""
            out = hpool.tile([128, 2, D], BF, tag="h")
            for tt in range(2):
                st = small.tile([128, 6], FP, tag="bnst")
                nc.vector.bn_stats(st[:], src[:, tt, :])
                mv = small.tile([128, 2], FP, tag="bnmv")
                nc.vector.bn_aggr(mv[:], st[:])
                nc.scalar.activation(mv[:, 1:2], mv[:, 1:2], AF.Sqrt,
                                     bias=epst[:, 0:1])
                nc.vector.reciprocal(mv[:, 1:2], mv[:, 1:2])
                nc.vector.tensor_scalar(
                    out=out[:, tt, :], in0=src[:, tt, :],
                    scalar1=mv[:, 0:1], scalar2=mv[:, 1:2],
                    op0=OP.subtract, op1=OP.mult)
            return out

        def transpose2(src, pool=t4):
            """src [128,2,D] bf16 (tokens, dims) -> [128,KD,CH] (dims, toks)."""
            out = pool.tile([128, KD, CH], BF, tag="t4" if pool is t4 else "xft")
            for d in range(KD):
                for tt in range(2):
                    ps = mmp.tile([128, 1024], BF, tag="mm")
                    nc.tensor.transpose(ps[:, 0:128], src[:, tt, ds(d * 128, 128)],
                                        ident[:])
                    nc.vector.tensor_copy(out[:, d, ds(tt * 128, 128)],
                                          ps[:, 0:128])
            return out

        nxt_wqkvo = nxt_w1h = nxt_w2h = None
        for l in range(layers):
            # ---- prefetch next layer's weights + one Wlm slice ----
            if l + 1 < layers:
                nxt_wqkvo = load_wqkvo(l + 1)
                nxt_w1h = load_w1h(l + 1, 0)
                nxt_w2h = load_w2h(l + 1, 0)
            if l < KD - 1:
                nc.gpsimd.dma_start(wlm_sb[:, l + 1, :], wlm[:, l + 1, :])

            # ---- LN1 + transpose ----
            h = layernorm(xt)
            hT = transpose2(h)

            # ---- k/v first so the AllGather starts ASAP ----
            kT = t4.tile([128, KD, CH], BF, tag="t4")
            for d in range(KD):
                ps = mmp.tile([128, 512], FP, tag="mm")
                for k in range(KD):
                    nc.tensor.matmul(ps[:, :CH], cur_wqkvo[:, k, 1, ds(d * 128, 128)],
                                     hT[:, k, :], start=(k == 0),
                                     stop=(k == KD - 1))
                if with_bias:
                    nc.vector.tensor_scalar_add(kT[:, d, :], ps[:, :CH],
                                                bqk_sb[:, l, 1, d:d + 1])
                else:
                    nc.vector.tensor_copy(kT[:, d, :], ps[:, :CH])
            vaug = vaugp.tile([128, 2, H, 65], BF)
            nc.vector.memset(vaug[:, :, :, 64:65], 1.0)
            for tt in range(2):
                ps = mmp.tile([128, 512], FP, tag="mm")
                for k in range(KD):
                    nc.tensor.matmul(ps[:, :D], hT[:, k, ds(tt * 128, 128)],
                                     cur_wqkvo[:, k, 2, :], start=(k == 0),
                                     stop=(k == KD - 1))
                nc.vector.tensor_copy(
                    vaug[:, tt, :, 0:64],
                    ps[:].rearrange("p (h e) -> p h e", h=H))

            # ---- AllGather K,V across batch group (bf16, split queues) ----
            kv_in = dram.tile([KV], BF, tag="kvin")
            nc.sync.dma_start(
                kv_in[0:KV_K].rearrange("(p a b) -> p a b", p=128, a=KD), kT[:])
            nc.scalar.dma_start(
                kv_in[KV_K:KV].rearrange("(p a h e) -> p a h e", p=128, a=2, h=H),
                vaug[:])
            kv_out = dram.tile([4, KV], BF, tag="kvout")
            nc.gpsimd.collective_compute(
                "AllGather", OP.bypass,
                replica_groups=[[0, 1, 2, 3], [4, 5, 6, 7]],
                ins=[kv_in[:].opt()], outs=[kv_out[:].opt()])

            # ---- q projection overlaps the collective ----
            qT = t4.tile([128, KD, CH], BF, tag="t4")
            for d in range(KD):
                ps = mmp.tile([128, 512], FP, tag="mm")
                for k in range(KD):
                    nc.tensor.matmul(ps[:, :CH], cur_wqkvo[:, k, 0, ds(d * 128, 128)],
                                     hT[:, k, :], start=(k == 0),
                                     stop=(k == KD - 1))
                if with_bias:
                    nc.vector.tensor_scalar_add(qT[:, d, :], ps[:, :CH],
                                                bqk_sb[:, l, 0, d:d + 1])
                else:
                    nc.vector.tensor_copy(qT[:, d, :], ps[:, :CH])

            kTall = kvall.tile([128, KD, 4, CH], BF, tag="ktall")
            vall = kvall.tile([128, 4, 2, H, 65], BF, tag="vall")
            for c in range(4):
                nc.sync.dma_start(
                    kTall[:, :, c, :],
                    kv_out[c, 0:KV_K].rearrange("(p a b) -> p a b", p=128, a=KD))
                nc.scalar.dma_start(
                    vall[:, c, :, :, :],
                    kv_out[c, KV_K:KV].rearrange("(p a h e) -> p a h e",
                                                 p=128, a=2, h=H))

            # ---- attention: per-head PSUM accumulation, pipelined norm ----
            attT = t4.tile([128, KD, CH], BF, tag="t4")
            dnr = attp.tile([128, H, CH], BF, tag="dnr", name=f"dnr{l}")
            rdram = dram.tile([H, CH], BF, tag="rdram")
            for hh in range(H):
                pb = (hh % 2) * 64
                dt_ = hh // 2
                avps = avp.tile([65, CH], FP, tag="av")
                for c in range(4):
                    sps = spp.tile([128, 2, CH], FP, tag="sp")
                    for kt in range(2):
                        nc.tensor.matmul(
                            sps[:, kt, :],
                            kTall[pb:pb + 64, dt_, c, ds(kt * 128, 128)],
                            qT[pb:pb + 64, dt_, :],
                            start=True, stop=True)
                    ex = expp.tile([128, 2, CH], BF, tag="exp")
                    nc.scalar.activation(ex[:], sps[:], AF.Exp)
                    nc.vector.tensor_tensor(ex[:], ex[:], msk_sb[:, c, :, :],
                                            OP.mult)
                    for kt in range(2):
                        nc.tensor.matmul(
                            avps[:], vall[:, c, kt, hh, :], ex[:, kt, :],
                            start=(c == 0 and kt == 0),
                            stop=(c == 3 and kt == 1))
                # denominator row -> DMA broadcast -> reciprocal -> scale
                nc.vector.tensor_copy(dnr[64:65, hh, :], avps[64:65, :])
                nc.sync.dma_start(rdram[hh, :], dnr[64:65, hh, :])
                rbc = rbcp.tile([64, CH], BF, tag="rbc")
                nc.sync.dma_start(
                    rbc[:],
                    bass.AP(tensor=rdram.tensor, offset=rdram.offset + hh * CH,
                            ap=[[0, 64], [1, CH]]))
                with nc.allow_low_precision(reason="softmax denom recip bf16"):
                    nc.vector.reciprocal(rbc[:], rbc[:])
                if pb == 0:
                    # even heads land on partitions 0-63: normalize in place
                    nc.vector.tensor_tensor(attT[0:64, dt_, :], avps[0:64, :],
                                            rbc[:], OP.mult)
                else:
                    att_n = expp.tile([128, 2, CH], BF, tag="exp")
                    nc.vector.tensor_tensor(att_n[0:64, 0, :], avps[0:64, :],
                                            rbc[:], OP.mult)
                    nc.sync.dma_start(attT[64:128, dt_, :], att_n[0:64, 0, :])

            # ---- Wo + bias + residual ----
            if with_bias:
                bo_b = bcp.tile([128, D], FP, tag="bc")
                bo_src = bo2[l, 0]
                nc.sync.dma_start(bo_b[:], bass.AP(
                    tensor=bo_src.tensor, offset=bo_src.offset,
                    ap=[[0, 128]] + list(bo_src.ap)))
            for tt in range(2):
                ps = mmp.tile([128, 512], FP, tag="mm")
                for k in range(KD):
                    nc.tensor.matmul(ps[:, :D], attT[:, k, ds(tt * 128, 128)],
                                     cur_wqkvo[:, k, 3, :], start=(k == 0),
                                     stop=(k == KD - 1))
                if with_bias:
                    nc.vector.tensor_tensor(ps[:, :D], ps[:, :D], bo_b[:],
                                            OP.add)
                nc.vector.tensor_tensor(xt[:, tt, :], xt[:, tt, :], ps[:, :D],
                                        OP.add)

            # ---- LN2 + transpose ----
            h2 = layernorm(xt)
            h2T = transpose2(h2)

            # ---- MLP ----
            if with_bias:
                b2_b = bcp.tile([128, D], FP, tag="bc")
                b2_src = bo2[l, 1]
                nc.sync.dma_start(b2_b[:], bass.AP(
                    tensor=b2_src.tensor, offset=b2_src.offset,
                    ap=[[0, 128]] + list(b2_src.ap)))
            x2ps = [mop.tile([128, D], FP, tag="mo", name=f"mo{l}_{kk}")
                    for kk in range(2)]
            for mp in range(MD // 2):
                gps = mmp.tile([128, 2, CH], FP, tag="mm")
                for mi in range(2):
                    m = mp * 2 + mi
                    w1h = cur_w1[m // (MD // 2)]
                    mm = m % (MD // 2)
                    for k in range(KD):
                        nc.tensor.matmul(gps[:, mi, :],
                                         w1h[:, k, ds(mm * 128, 128)],
                                         h2T[:, k, :], start=(k == 0),
                                         stop=(k == KD - 1))
                gt = gtp.tile([128, 2, CH], BF, tag="gt")
                if with_bias:
                    for mi in range(2):
                        m = mp * 2 + mi
                        nc.scalar.activation(gt[:, mi, :], gps[:, mi, :], AF.Gelu,
                                             bias=b1_sb[:, l, m:m + 1])
                else:
                    nc.scalar.activation(gt[:], gps[:], AF.Gelu)
                for mi in range(2):
                    m = mp * 2 + mi
                    w2h = cur_w2[m // (MD // 2)]
                    for tt in range(2):
                        nc.tensor.matmul(x2ps[tt][:], gt[:, mi, ds(tt * 128, 128)],
                                         w2h[:, m % (MD // 2), :],
                                         start=(m == 0), stop=(m == MD - 1))
                if mp == 3 and l + 1 < layers:
                    # the current layer's first halves had their last read at
                    # m=7 (this iteration) - their ring slots are now free
                    nxt_w1 = [nxt_w1h, load_w1h(l + 1, 1)]
                    nxt_w2 = [nxt_w2h, load_w2h(l + 1, 1)]
            for tt in range(2):
                if with_bias:
                    nc.vector.tensor_tensor(x2ps[tt][:], x2ps[tt][:], b2_b[:],
                                            OP.add)
                nc.vector.tensor_tensor(xt[:, tt, :], xt[:, tt, :], x2ps[tt][:],
                                        OP.add)
            if l + 1 < layers:
                cur_wqkvo, cur_w1, cur_w2 = nxt_wqkvo, nxt_w1, nxt_w2

        # ---- final LN; AllGather hidden state across all 8 (Shared/RDH) ----
        xf = layernorm(xt)
        xfT = transpose2(xf, pool=xftp)
        xf_in = dram.tile([XF], BF, tag="xfin")
        nc.sync.dma_start(
            xf_in[:].rearrange("(p a b) -> p a b", p=128, a=KD), xfT[:])
        xf_out = dram.tile([NC, XF], BF, tag="xfout", addr_space="Shared")
        nc.gpsimd.collective_compute(
            "AllGather", OP.bypass,
            replica_groups=[list(range(NC))],
            ins=[xf_in[:].opt()], outs=[xf_out[:].opt()])

        hp_pools = ((mmp, "mm"), (mop, "mo"), (spp, "sp"))
        dma_engs = (nc.sync, nc.scalar, nc.gpsimd)
        gi = 0
        LGB = [0, 5, 9, 13]  # logits go out in chunks of 5/4/4 vocab tiles

        def head_group(get_lhs, tg):
            """One 128-token group: all NT vocab tiles -> three chunked DMAs.
            tg None => warmup-only (compute and discard)."""
            nonlocal gi
            lgt = None
            cj = 0
            for n in range(NT):
                if tg is not None and n == LGB[cj]:
                    lgt = lgp.tile([128, 5, 512], BF, tag="lg",
                                   name=f"lg{tg}_{cj}")
                nsz = min(512, VS - n * 512)
                pool_i, ptag = hp_pools[gi % 3]
                ps = pool_i.tile([128, 512], FP, tag=ptag,
                                 name=f"hps{tg}_{n}")
                for k in range(KD):
                    nc.tensor.matmul(
                        ps[:, :nsz], get_lhs(k),
                        wlm_sb[:, k, ds(n * 512, nsz)],
                        start=(k == 0), stop=(k == KD - 1))
                gi += 1
                if tg is None:
                    continue
                nb = n - LGB[cj]
                if gi % 2 == 0:
                    nc.vector.tensor_copy(lgt[:, nb, :nsz], ps[:, :nsz])
                else:
                    nc.scalar.activation(lgt[:, nb, :nsz], ps[:, :nsz], AF.Copy)
                if n == LGB[cj + 1] - 1:
                    nw = LGB[cj + 1] - LGB[cj]
                    dma_engs[(tg + cj) % 3].dma_start(
                        logits[tg, :, ds(LGB[cj] * 512, nw * 512)],
                        lgt[:, 0:nw, :])
                    cj += 1

        # pass 1: warmup on local tokens while the AllGather flies (discard)
        for mt in range(2):
            head_group(lambda k, mt=mt: xfT[:, k, ds(mt * 128, 128)], None)

        # read the gathered slots back (alternate queues), then pass 2
        xfall = []
        for cg in range(2):
            xa = xfap.tile([128, KD, 4, CH], BF, tag="xfa", name=f"xfa{cg}")
            for c in range(4):
                dma_engs[(cg * 4 + c) % 3].dma_start(
                    xa[:, :, c, :],
                    xf_out[cg * 4 + c, :].rearrange("(p a b) -> p a b",
                                                    p=128, a=KD))
            xfall.append(xa)
        for c in range(NC):
            for mt in range(2):
                head_group(
                    lambda k, c=c, mt=mt:
                        xfall[c // 4][:, k, c % 4, ds(mt * 128, 128)],
                    c * 2 + mt)

    nc.compile()
    return nc


_CACHE = {}


def _get_program(with_bias=True):
    key = ("nc", with_bias)
    if key not in _CACHE:
        _CACHE[key] = build_program(with_bias=with_bias)
    return _CACHE[key]


def _to_pmaj(w):
    """[.., D_in, N] -> [.., 128, D_in//128, N] partition-major."""
    shp = w.shape
    kd = shp[-2] // 128
    return np.ascontiguousarray(
        w.reshape(*shp[:-2], kd, 128, shp[-1]).swapaxes(-3, -2))


def _prep_inputs(inputs):
    f = lambda k: np.asarray(inputs[k], np.float32)
    bf = ml_dtypes.bfloat16
    idx = np.asarray(inputs["idx"]).astype(np.int64)
    tok_emb, pos_emb = f("tok_emb"), f("pos_emb")
    x0 = tok_emb[idx] + pos_emb[None, :T]          # [B, T, D]
    x0 = x0.reshape(NC, 2, 128, D).transpose(0, 2, 1, 3).copy()  # [NC,128,2,D]

    ln1_g, ln1_b = f("ln1_g"), f("ln1_b")
    ln2_g, ln2_b = f("ln2_g"), f("ln2_b")
    Wq, bq = f("Wq"), f("bq")
    Wk, bk = f("Wk"), f("bk")
    Wv, bv = f("Wv"), f("bv")
    Wo, bo = f("Wo"), f("bo")
    W1, b1 = f("W1"), f("b1")
    W2, b2 = f("W2"), f("b2")
    lnf_g, lnf_b = f("lnf_g"), f("lnf_b")
    Wlm, blm = f("Wlm"), f("blm")

    sc = 1.0 / np.sqrt(HS)
    wqe = ln1_g[:, :, None] * Wq * sc
    bqe = (np.einsum("ld,ldm->lm", ln1_b, Wq) + bq) * sc
    wke = ln1_g[:, :, None] * Wk
    bke = np.einsum("ld,ldm->lm", ln1_b, Wk) + bk
    wve = ln1_g[:, :, None] * Wv
    bve = np.einsum("ld,ldm->lm", ln1_b, Wv) + bv
    boe = np.einsum("lm,lmd->ld", bve, Wo) + bo
    w1e = ln2_g[:, :, None] * W1
    b1e = np.einsum("ld,ldf->lf", ln2_b, W1) + b1
    wlme = lnf_g[:, None] * Wlm
    blme = lnf_b @ Wlm + blm

    bqk = np.stack([bqe, bke], axis=1)             # [L, 2, D]
    bqk = bqk.reshape(L, 2, KD, 128).transpose(3, 0, 1, 2).copy()
    b1t = b1e.reshape(L, MD, 128).transpose(2, 0, 1).copy()
    bo2 = np.stack([boe, b2], axis=1)              # [L, 2, D]

    wlmp = np.zeros((D, NC * VS), np.float32)
    wlmp[:, :V] = wlme

    masks = []
    for core in range(NC):
        cc = core % 4
        qpos = cc * CH + np.arange(CH)
        m = np.empty((128, 4, 2, CH), np.float32)
        for kc in range(4):
            for kt in range(2):
                kpos = kc * CH + kt * 128 + np.arange(128)
                m[:, kc, kt, :] = (kpos[:, None] <= qpos[None, :]).astype(np.float32)
        masks.append(m.astype(bf))

    shared = dict(wq=_to_pmaj(wqe.astype(bf)), wk=_to_pmaj(wke.astype(bf)),
                  wv=_to_pmaj(wve.astype(bf)), wo=_to_pmaj(Wo.astype(bf)),
                  w1=_to_pmaj(w1e.astype(bf)), w2=_to_pmaj(W2.astype(bf)),
                  bqk=bqk, b1t=b1t, bo2=np.ascontiguousarray(bo2))
    in_maps = []
    for core in range(NC):
        m = dict(shared)
        m["x0"] = np.ascontiguousarray(x0[core])
        m["msk"] = masks[core]
        m["wlm"] = _to_pmaj(wlmp[:, core * VS:(core + 1) * VS].astype(bf))
        in_maps.append(m)
    return in_maps, blme


def _run(inputs, trace=False):
    in_maps, blme = _prep_inputs(inputs)
    with_bias = bool(np.any(in_maps[0]["bo2"]))
    nc = _get_program(with_bias=with_bias)
    res = bass_utils.run_bass_kernel_spmd(nc, in_maps, core_ids=list(range(NC)),
                                          trace=trace)
    lg = np.concatenate(
        [np.asarray(res.results[c]["logits"]).astype(np.float32)
         .reshape(BT, NTP)[:, :VS] for c in range(NC)], axis=1)
    out = lg[:, :V]
    if np.any(blme):
        out = out + blme[None, :]
    return out.reshape(B, T, V).astype(np.float32), res


def kernel(**inputs) -> np.ndarray:
    out, _ = _run(inputs, trace=False)
    return out


# revision 23
# speedup vs baseline: 1.1010x; 1.1010x over previous
"""Trainium2 Bass kernel for a 6-layer GPT (D=512, H=8, T=1024, B=2, V=50257).

Strategy (8 NeuronCores), v3:
- Token-shard the transformer body: core c owns 256 tokens (cores 0-3 =
  batch 0 chunks 0-3, cores 4-7 = batch 1 chunks 0-3).
- All matmul operands bf16 (PSUM accumulates fp32); residual/LN fp32.
- Host pre-rearranges every weight into partition-major layout so DMA
  descriptors are >=4KB contiguous lines; DMAs are spread over the three
  trigger queues (sync / scalar / gpsimd).
- Per layer: LN1 -> QKV -> AllGather K,V (bf16) across the batch group
  (next layer's weights prefetched at layer start) -> causal attention,
  per-head PSUM accumulation, masks multiplied on the vector engine,
  per-head pipelined softmax normalization -> Wo + residual -> LN2 ->
  MLP -> residual.
- Final LN -> AllGather hidden (8-core, Shared/RDH) overlapped with
  discarded warmup matmuls on the local tokens -> vocab-sharded LM head
  with Wlm resident in SBUF (loaded in 4 slices during layers 0-2);
  logits accumulate 13 vocab tiles in SBUF and go out as one 13KB-per-
  partition DMA per 128-token group, alternating queues.
- Host folds LN gamma/beta and the 1/sqrt(HS) score scale into the
  weights; embedding gather happens host-side (tiny).
"""

import numpy as np
import ml_dtypes

import concourse.bass as bass
import concourse.tile as tile
from concourse import bacc, mybir
from concourse import bass_utils
from concourse.bass import ds, ts
from concourse.masks import make_identity

FP = mybir.dt.float32
BF = mybir.dt.bfloat16
AF = mybir.ActivationFunctionType
OP = mybir.AluOpType

V, D, T, L, H, HS, B = 50257, 512, 1024, 6, 8, 64, 2
FF = 4 * D
EPS = 1e-5
NC = 8          # cores
CH = 256        # tokens per core
VS = 6284       # padded vocab shard per core; 8*VS = 50272 >= V
KD = D // 128   # 4 k-tiles over D
MD = FF // 128  # 16 m-tiles over FF
BT = B * T
NT = (VS + 511) // 512   # 13 vocab tiles per core
NTP = NT * 512           # padded vocab shard (6656)


def build_program(with_bias=True, layers=L):
    nc = bacc.Bacc("TRN2", target_bir_lowering=False, debug=False, num_devices=NC)

    # ---- I/O (weights host-side pre-rearranged to partition-major) ----
    x0 = nc.dram_tensor("x0", [128, 2, D], FP, kind="ExternalInput").ap()
    wq = nc.dram_tensor("wq", [L, 128, KD, D], BF, kind="ExternalInput").ap()
    wk = nc.dram_tensor("wk", [L, 128, KD, D], BF, kind="ExternalInput").ap()
    wv = nc.dram_tensor("wv", [L, 128, KD, D], BF, kind="ExternalInput").ap()
    wo = nc.dram_tensor("wo", [L, 128, KD, D], BF, kind="ExternalInput").ap()
    w1 = nc.dram_tensor("w1", [L, 128, KD, FF], BF, kind="ExternalInput").ap()
    w2 = nc.dram_tensor("w2", [L, 128, MD, D], BF, kind="ExternalInput").ap()
    wlm = nc.dram_tensor("wlm", [128, KD, VS], BF, kind="ExternalInput").ap()
    bqk = nc.dram_tensor("bqk", [128, L, 2, KD], FP, kind="ExternalInput").ap()
    b1t = nc.dram_tensor("b1t", [128, L, MD], FP, kind="ExternalInput").ap()
    bo2 = nc.dram_tensor("bo2", [L, 2, D], FP, kind="ExternalInput").ap()
    # causal 0/1 mask per core: [p, kchunk, ktile, 256 queries] bf16
    msk = nc.dram_tensor("msk", [128, 4, 2, CH], BF, kind="ExternalInput").ap()
    # logits row (tg, p) = token tg*128+p of the gathered order
    logits = nc.dram_tensor("logits", [BT // 128, 128, NTP], BF,
                            kind="ExternalOutput").ap()

    KV_K = 128 * KD * CH            # kT flat elements per core
    KV_V = 128 * 2 * H * 65         # v_aug flat elements per core
    KV = KV_K + KV_V
    XF = 128 * KD * CH              # xfT flat elements

    from contextlib import ExitStack
    with ExitStack() as stk:
        tc = stk.enter_context(tile.TileContext(nc))
        ec = stk.enter_context
        consts = ec(tc.tile_pool(name="consts", bufs=1))
        wlmp = ec(tc.tile_pool(name="wlmp", bufs=1))
        xpool = ec(tc.tile_pool(name="xpool", bufs=1))
        hpool = ec(tc.tile_pool(name="hpool", bufs=1))
        t4 = ec(tc.tile_pool(name="t4", bufs=4))          # [128,KD,CH] transposed acts
        wqkvop = ec(tc.tile_pool(name="wqkvo", bufs=2))   # [128,KD,4,512]
        w1pool = ec(tc.tile_pool(name="w1k", bufs=3))     # [128,KD,FF//2] halves
        w2pool = ec(tc.tile_pool(name="w2k", bufs=3))     # [128,MD//2,D] halves
        kvall = ec(tc.tile_pool(name="kvall", bufs=1))
        vaugp = ec(tc.tile_pool(name="vaug", bufs=1))
        small = ec(tc.tile_pool(name="small", bufs=2))
        expp = ec(tc.tile_pool(name="exp", bufs=2))
        gtp = ec(tc.tile_pool(name="gt", bufs=2))
        attp = ec(tc.tile_pool(name="attp", bufs=1))
        rbcp = ec(tc.tile_pool(name="rbc", bufs=4))
        lgp = ec(tc.tile_pool(name="lg", bufs=2))
        bcp = ec(tc.tile_pool(name="bcast", bufs=2))
        xftp = ec(tc.tile_pool(name="xft", bufs=1))
        xfap = ec(tc.tile_pool(name="xfa", bufs=2))
        mmp = ec(tc.tile_pool(name="mm", bufs=2, space="PSUM"))
        avp = ec(tc.tile_pool(name="avp", bufs=2, space="PSUM"))
        spp = ec(tc.tile_pool(name="sp", bufs=2, space="PSUM"))
        mop = ec(tc.tile_pool(name="mo", bufs=2, space="PSUM"))
        dram = ec(tc.tile_pool(name="dram", bufs=2, space="DRAM"))

        # ---- consts + first activations on the sync queue ----
        xt = xpool.tile([128, 2, D], FP, tag="xt")
        nc.sync.dma_start(xt[:], x0)
        ident = consts.tile([128, 128], BF)
        make_identity(nc, ident)
        epst = consts.tile([128, 1], FP)
        nc.vector.memset(epst, EPS)
        msk_sb = consts.tile([128, 4, 2, CH], BF)
        nc.sync.dma_start(msk_sb[:], msk)
        bqk_sb = consts.tile([128, L, 2, KD], FP)
        b1_sb = consts.tile([128, L, MD], FP)
        if with_bias:
            nc.sync.dma_start(bqk_sb[:], bqk)
            nc.sync.dma_start(b1_sb[:], b1t)

        # ---- layer-0 weights (scalar + gpsimd queues), then resident Wlm --
        def load_wqkvo(l):
            w = wqkvop.tile([128, KD, 4, 512], BF, tag="wqkvo", name=f"wqkvo{l}")
            for i, src in enumerate((wq, wk, wv, wo)):
                eng = nc.scalar if i < 2 else nc.gpsimd
                eng.dma_start(w[:, :, i, :], src[l])
            return w

        def load_w1h(l, i):
            w = w1pool.tile([128, KD, FF // 2], BF, tag="w1k",
                            name=f"w1k{l}_{i}")
            nc.scalar.dma_start(w[:], w1[l, :, :, ds(i * (FF // 2), FF // 2)])
            return w

        def load_w2h(l, i):
            w = w2pool.tile([128, MD // 2, D], BF, tag="w2k",
                            name=f"w2k{l}_{i}")
            nc.gpsimd.dma_start(w[:], w2[l, :, ds(i * (MD // 2), MD // 2), :])
            return w

        cur_wqkvo = load_wqkvo(0)
        cur_w1 = [load_w1h(0, 0), load_w1h(0, 1)]
        cur_w2 = [load_w2h(0, 0), load_w2h(0, 1)]
        wlm_sb = wlmp.tile([128, KD, VS], BF)
        nc.gpsimd.dma_start(wlm_sb[:, 0, :], wlm[:, 0, :])

        def layernorm(src):
            """src [128,2,D] fp32 -> normalized [128,2,D] bf16."""
            out = hpool.tile([128, 2, D], BF, tag="h")
            for tt in range(2):
                st = small.tile([128, 6], FP, tag="bnst")
                nc.vector.bn_stats(st[:], src[:, tt, :])
                mv = small.tile([128, 2], FP, tag="bnmv")
                nc.vector.bn_aggr(mv[:], st[:])
                nc.scalar.activation(mv[:, 1:2], mv[:, 1:2], AF.Sqrt,
                                     bias=epst[:, 0:1])
                nc.vector.reciprocal(mv[:, 1:2], mv[:, 1:2])
                nc.vector.tensor_scalar(
                    out=out[:, tt, :], in0=src[:, tt, :],
                    scalar1=mv[:, 0:1], scalar2=mv[:, 1:2],
                    op0=OP.subtract, op1=OP.mult)
            return out

        def transpose2(src, pool=t4):
            """src [128,2,D] bf16 (tokens, dims) -> [128,KD,CH] (dims, toks)."""
            out = pool.tile([128, KD, CH], BF, tag="t4" if pool is t4 else "xft")
            for d in range(KD):
                for tt in range(2):
                    ps = mmp.tile([128, 1024], BF, tag="mm")
                    nc.tensor.transpose(ps[:, 0:128], src[:, tt, ds(d * 128, 128)],
                                        ident[:])
                    nc.vector.tensor_copy(out[:, d, ds(tt * 128, 128)],
                                          ps[:, 0:128])
            return out

        nxt_wqkvo = nxt_w1h = nxt_w2h = None
        for l in range(layers):
            # ---- prefetch next layer's weights + one Wlm slice ----
            if l + 1 < layers:
                nxt_wqkvo = load_wqkvo(l + 1)
                nxt_w1h = load_w1h(l + 1, 0)
                nxt_w2h = load_w2h(l + 1, 0)
            if l < KD - 1:
                nc.gpsimd.dma_start(wlm_sb[:, l + 1, :], wlm[:, l + 1, :])

            # ---- LN1 + transpose ----
            h = layernorm(xt)
            hT = transpose2(h)

            # ---- k/v first so the AllGather starts ASAP ----
            kT = t4.tile([128, KD, CH], BF, tag="t4")
            for d in range(KD):
                ps = mmp.tile([128, 512], FP, tag="mm")
                for k in range(KD):
                    nc.tensor.matmul(ps[:, :CH], cur_wqkvo[:, k, 1, ds(d * 128, 128)],
                                     hT[:, k, :], start=(k == 0),
                                     stop=(k == KD - 1))
                if with_bias:
                    nc.vector.tensor_scalar_add(kT[:, d, :], ps[:, :CH],
                                                bqk_sb[:, l, 1, d:d + 1])
                else:
                    nc.vector.tensor_copy(kT[:, d, :], ps[:, :CH])
            vaug = vaugp.tile([128, 2, H, 65], BF)
            nc.vector.memset(vaug[:, :, :, 64:65], 1.0)
            for tt in range(2):
                ps = mmp.tile([128, 512], FP, tag="mm")
                for k in range(KD):
                    nc.tensor.matmul(ps[:, :D], hT[:, k, ds(tt * 128, 128)],
                                     cur_wqkvo[:, k, 2, :], start=(k == 0),
                                     stop=(k == KD - 1))
                nc.vector.tensor_copy(
                    vaug[:, tt, :, 0:64],
                    ps[:].rearrange("p (h e) -> p h e", h=H))

            # ---- AllGather K,V across batch group (bf16, split queues) ----
            kv_in = dram.tile([KV], BF, tag="kvin")
            nc.sync.dma_start(
                kv_in[0:KV_K].rearrange("(p a b) -> p a b", p=128, a=KD), kT[:])
            nc.scalar.dma_start(
                kv_in[KV_K:KV].rearrange("(p a h e) -> p a h e", p=128, a=2, h=H),
                vaug[:])
            kv_out = dram.tile([4, KV], BF, tag="kvout")
            nc.gpsimd.collective_compute(
                "AllGather", OP.bypass,
                replica_groups=[[0, 1, 2, 3], [4, 5, 6, 7]],
                ins=[kv_in[:].opt()], outs=[kv_out[:].opt()])

            # ---- q projection overlaps the collective ----
            qT = t4.tile([128, KD, CH], BF, tag="t4")
            for d in range(KD):
                ps = mmp.tile([128, 512], FP, tag="mm")
                for k in range(KD):
                    nc.tensor.matmul(ps[:, :CH], cur_wqkvo[:, k, 0, ds(d * 128, 128)],
                                     hT[:, k, :], start=(k == 0),
                                     stop=(k == KD - 1))
                if with_bias:
                    nc.vector.tensor_scalar_add(qT[:, d, :], ps[:, :CH],
                                                bqk_sb[:, l, 0, d:d + 1])
                else:
                    nc.vector.tensor_copy(qT[:, d, :], ps[:, :CH])

            kTall = kvall.tile([128, KD, 4, CH], BF, tag="ktall")
            vall = kvall.tile([128, 4, 2, H, 65], BF, tag="vall")
            for c in range(4):
                nc.sync.dma_start(
                    kTall[:, :, c, :],
                    kv_out[c, 0:KV_K].rearrange("(p a b) -> p a b", p=128, a=KD))
                nc.scalar.dma_start(
                    vall[:, c, :, :, :],
                    kv_out[c, KV_K:KV].rearrange("(p a h e) -> p a h e",
                                                 p=128, a=2, h=H))

            # ---- attention: per-head PSUM accumulation over the 4 chunks.
            # PSUM is drained eagerly; softmax denominators are processed in
            # two groups of 4 heads via a compact reciprocal so every DVE
            # wait lands in an already-covered window.
            attT = t4.tile([128, KD, CH], BF, tag="t4")
            # araw rows 0-63 = numerators; row 64 = denominators
            araw = attp.tile([128, H, CH], BF, tag="araw", name=f"araw{l}")
            rdram = dram.tile([2, 4 * CH], BF, tag="rdram")
            rdram2 = dram.tile([2, 4 * CH], BF, tag="rdram2")
            rbcs = {}

            def denom_compact(g):
                """denominators of head group g (4 heads) -> reciprocals."""
                rsb = small.tile([128, 4 * CH // 128], BF, tag="rsb")
                nc.sync.dma_start(
                    rsb[:], rdram[g].rearrange("(p f) -> p f", p=128))
                with nc.allow_low_precision(reason="softmax denom recip bf16"):
                    nc.vector.reciprocal(rsb[:], rsb[:])
                nc.sync.dma_start(
                    rdram2[g].rearrange("(p f) -> p f", p=128), rsb[:])
                for hh in range(g * 4, g * 4 + 4):
                    rbc = rbcp.tile([64, CH], BF, tag="rbc", name=f"rb{l}_{hh}")
                    nc.sync.dma_start(
                        rbc[:],
                        bass.AP(tensor=rdram2.tensor,
                                offset=rdram2.offset + g * 4 * CH
                                + (hh % 4) * CH,
                                ap=[[0, 64], [1, CH]]))
                    rbcs[hh] = rbc

            def norm_head(hh):
                pb = (hh % 2) * 64
                dt_ = hh // 2
                if pb == 0:
                    nc.vector.tensor_tensor(attT[0:64, dt_, :],
                                            araw[0:64, hh, :],
                                            rbcs[hh][:], OP.mult)
                else:
                    att_n = expp.tile([128, 2, CH], BF, tag="exp")
                    nc.vector.tensor_tensor(att_n[0:64, 0, :],
                                            araw[0:64, hh, :],
                                            rbcs[hh][:], OP.mult)
                    nc.sync.dma_start(attT[64:128, dt_, :], att_n[0:64, 0, :])

            for hh in range(H):
                pb = (hh % 2) * 64
                dt_ = hh // 2
                avps = avp.tile([65, CH], FP, tag="av")
                for c in range(4):
                    sps = spp.tile([128, 2, CH], FP, tag="sp")
                    for kt in range(2):
                        nc.tensor.matmul(
                            sps[:, kt, :],
                            kTall[pb:pb + 64, dt_, c, ds(kt * 128, 128)],
                            qT[pb:pb + 64, dt_, :],
                            start=True, stop=True)
                    ex = expp.tile([128, 2, CH], BF, tag="exp")
                    nc.scalar.activation(ex[:], sps[:], AF.Exp)
                    # alternate mask engine so neither becomes the straggler
                    if c % 2 == 0:
                        nc.vector.tensor_tensor(ex[:], ex[:], msk_sb[:, c, :, :],
                                                OP.mult)
                    else:
                        nc.gpsimd.tensor_tensor(ex[:], ex[:], msk_sb[:, c, :, :],
                                                OP.mult)
                    for kt in range(2):
                        nc.tensor.matmul(
                            avps[:], vall[:, c, kt, hh, :], ex[:, kt, :],
                            start=(c == 0 and kt == 0),
                            stop=(c == 3 and kt == 1))
                # eagerly drain PSUM so later heads are never blocked
                nc.vector.tensor_copy(araw[0:65, hh, :], avps[0:65, :])
                nc.sync.dma_start(
                    rdram[hh // 4].rearrange("(a b) -> a b", a=4)
                    [hh % 4:hh % 4 + 1, :],
                    araw[64:65, hh, :])
                if hh == 3:
                    denom_compact(0)
                if hh == 5:
                    for h2 in range(4):
                        norm_head(h2)
            denom_compact(1)
            for h2 in range(4, 8):
                norm_head(h2)

            # ---- Wo + bias + residual ----
            if with_bias:
                bo_b = bcp.tile([128, D], FP, tag="bc")
                bo_src = bo2[l, 0]
                nc.sync.dma_start(bo_b[:], bass.AP(
                    tensor=bo_src.tensor, offset=bo_src.offset,
                    ap=[[0, 128]] + list(bo_src.ap)))
            for tt in range(2):
                ps = mmp.tile([128, 512], FP, tag="mm")
                for k in range(KD):
                    nc.tensor.matmul(ps[:, :D], attT[:, k, ds(tt * 128, 128)],
                                     cur_wqkvo[:, k, 3, :], start=(k == 0),
                                     stop=(k == KD - 1))
                if with_bias:
                    nc.vector.tensor_tensor(ps[:, :D], ps[:, :D], bo_b[:],
                                            OP.add)
                nc.vector.tensor_tensor(xt[:, tt, :], xt[:, tt, :], ps[:, :D],
                                        OP.add)

            # ---- LN2 + transpose ----
            h2 = layernorm(xt)
            h2T = transpose2(h2)

            # ---- MLP ----
            if with_bias:
                b2_b = bcp.tile([128, D], FP, tag="bc")
                b2_src = bo2[l, 1]
                nc.sync.dma_start(b2_b[:], bass.AP(
                    tensor=b2_src.tensor, offset=b2_src.offset,
                    ap=[[0, 128]] + list(b2_src.ap)))
            x2ps = [mop.tile([128, D], FP, tag="mo", name=f"mo{l}_{kk}")
                    for kk in range(2)]
            for mp in range(MD // 2):
                gps = mmp.tile([128, 2, CH], FP, tag="mm")
                for mi in range(2):
                    m = mp * 2 + mi
                    w1h = cur_w1[m // (MD // 2)]
                    mm = m % (MD // 2)
                    for k in range(KD):
                        nc.tensor.matmul(gps[:, mi, :],
                                         w1h[:, k, ds(mm * 128, 128)],
                                         h2T[:, k, :], start=(k == 0),
                                         stop=(k == KD - 1))
                gt = gtp.tile([128, 2, CH], BF, tag="gt")
                if with_bias:
                    for mi in range(2):
                        m = mp * 2 + mi
                        nc.scalar.activation(gt[:, mi, :], gps[:, mi, :], AF.Gelu,
                                             bias=b1_sb[:, l, m:m + 1])
                else:
                    nc.scalar.activation(gt[:], gps[:], AF.Gelu)
                for mi in range(2):
                    m = mp * 2 + mi
                    w2h = cur_w2[m // (MD // 2)]
                    for tt in range(2):
                        nc.tensor.matmul(x2ps[tt][:], gt[:, mi, ds(tt * 128, 128)],
                                         w2h[:, m % (MD // 2), :],
                                         start=(m == 0), stop=(m == MD - 1))
                if mp == 3 and l + 1 < layers:
                    # the current layer's first halves had their last read at
                    # m=7 (this iteration) - their ring slots are now free
                    nxt_w1 = [nxt_w1h, load_w1h(l + 1, 1)]
                    nxt_w2 = [nxt_w2h, load_w2h(l + 1, 1)]
            for tt in range(2):
                if with_bias:
                    nc.vector.tensor_tensor(x2ps[tt][:], x2ps[tt][:], b2_b[:],
                                            OP.add)
                nc.vector.tensor_tensor(xt[:, tt, :], xt[:, tt, :], x2ps[tt][:],
                                        OP.add)
            if l + 1 < layers:
                cur_wqkvo, cur_w1, cur_w2 = nxt_wqkvo, nxt_w1, nxt_w2

        # ---- final LN; AllGather hidden state across all 8 (Shared/RDH) ----
        xf = layernorm(xt)
        xfT = transpose2(xf, pool=xftp)
        xf_in = dram.tile([XF], BF, tag="xfin")
        nc.sync.dma_start(
            xf_in[:].rearrange("(p a b) -> p a b", p=128, a=KD), xfT[:])
        xf_out = dram.tile([NC, XF], BF, tag="xfout", addr_space="Shared")
        nc.gpsimd.collective_compute(
            "AllGather", OP.bypass,
            replica_groups=[list(range(NC))],
            ins=[xf_in[:].opt()], outs=[xf_out[:].opt()])

        hp_pools = ((mmp, "mm"), (mop, "mo"), (spp, "sp"))
        dma_engs = (nc.sync, nc.scalar, nc.gpsimd)
        gi = 0
        LGB = [0, 5, 9, 13]  # logits go out in chunks of 5/4/4 vocab tiles

        def head_group(get_lhs, tg):
            """One 128-token group: all NT vocab tiles -> three chunked DMAs.
            tg None => warmup-only (compute and discard)."""
            nonlocal gi
            lgt = None
            cj = 0
            for n in range(NT):
                if tg is not None and n == LGB[cj]:
                    lgt = lgp.tile([128, 5, 512], BF, tag="lg",
                                   name=f"lg{tg}_{cj}")
                nsz = min(512, VS - n * 512)
                pool_i, ptag = hp_pools[gi % 3]
                ps = pool_i.tile([128, 512], FP, tag=ptag,
                                 name=f"hps{tg}_{n}")
                for k in range(KD):
                    nc.tensor.matmul(
                        ps[:, :nsz], get_lhs(k),
                        wlm_sb[:, k, ds(n * 512, nsz)],
                        start=(k == 0), stop=(k == KD - 1))
                gi += 1
                if tg is None:
                    continue
                nb = n - LGB[cj]
                if gi % 2 == 0:
                    nc.vector.tensor_copy(lgt[:, nb, :nsz], ps[:, :nsz])
                else:
                    nc.scalar.activation(lgt[:, nb, :nsz], ps[:, :nsz], AF.Copy)
                if n == LGB[cj + 1] - 1:
                    nw = LGB[cj + 1] - LGB[cj]
                    dma_engs[(tg + cj) % 3].dma_start(
                        logits[tg, :, ds(LGB[cj] * 512, nw * 512)],
                        lgt[:, 0:nw, :])
                    cj += 1

        # pass 1: warmup on local tokens while the AllGather flies (discard)
        for mt in range(2):
            head_group(lambda k, mt=mt: xfT[:, k, ds(mt * 128, 128)], None)

        # read the gathered slots back (alternate queues), then pass 2
        xfall = []
        for cg in range(2):
            xa = xfap.tile([128, KD, 4, CH], BF, tag="xfa", name=f"xfa{cg}")
            for c in range(4):
                dma_engs[(cg * 4 + c) % 3].dma_start(
                    xa[:, :, c, :],
                    xf_out[cg * 4 + c, :].rearrange("(p a b) -> p a b",
                                                    p=128, a=KD))
            xfall.append(xa)
        for c in range(NC):
            for mt in range(2):
                head_group(
                    lambda k, c=c, mt=mt:
                        xfall[c // 4][:, k, c % 4, ds(mt * 128, 128)],
                    c * 2 + mt)

    nc.compile()
    return nc


_CACHE = {}


def _get_program(with_bias=True):
    key = ("nc", with_bias)
    if key not in _CACHE:
        _CACHE[key] = build_program(with_bias=with_bias)
    return _CACHE[key]


def _to_pmaj(w):
    """[.., D_in, N] -> [.., 128, D_in//128, N] partition-major."""
    shp = w.shape
    kd = shp[-2] // 128
    return np.ascontiguousarray(
        w.reshape(*shp[:-2], kd, 128, shp[-1]).swapaxes(-3, -2))


def _prep_inputs(inputs):
    f = lambda k: np.asarray(inputs[k], np.float32)
    bf = ml_dtypes.bfloat16
    idx = np.asarray(inputs["idx"]).astype(np.int64)
    tok_emb, pos_emb = f("tok_emb"), f("pos_emb")
    x0 = tok_emb[idx] + pos_emb[None, :T]          # [B, T, D]
    x0 = x0.reshape(NC, 2, 128, D).transpose(0, 2, 1, 3).copy()  # [NC,128,2,D]

    ln1_g, ln1_b = f("ln1_g"), f("ln1_b")
    ln2_g, ln2_b = f("ln2_g"), f("ln2_b")
    Wq, bq = f("Wq"), f("bq")
    Wk, bk = f("Wk"), f("bk")
    Wv, bv = f("Wv"), f("bv")
    Wo, bo = f("Wo"), f("bo")
    W1, b1 = f("W1"), f("b1")
    W2, b2 = f("W2"), f("b2")
    lnf_g, lnf_b = f("lnf_g"), f("lnf_b")
    Wlm, blm = f("Wlm"), f("blm")

    sc = 1.0 / np.sqrt(HS)
    wqe = ln1_g[:, :, None] * Wq * sc
    bqe = (np.einsum("ld,ldm->lm", ln1_b, Wq) + bq) * sc
    wke = ln1_g[:, :, None] * Wk
    bke = np.einsum("ld,ldm->lm", ln1_b, Wk) + bk
    wve = ln1_g[:, :, None] * Wv
    bve = np.einsum("ld,ldm->lm", ln1_b, Wv) + bv
    boe = np.einsum("lm,lmd->ld", bve, Wo) + bo
    w1e = ln2_g[:, :, None] * W1
    b1e = np.einsum("ld,ldf->lf", ln2_b, W1) + b1
    wlme = lnf_g[:, None] * Wlm
    blme = lnf_b @ Wlm + blm

    bqk = np.stack([bqe, bke], axis=1)             # [L, 2, D]
    bqk = bqk.reshape(L, 2, KD, 128).transpose(3, 0, 1, 2).copy()
    b1t = b1e.reshape(L, MD, 128).transpose(2, 0, 1).copy()
    bo2 = np.stack([boe, b2], axis=1)              # [L, 2, D]

    wlmp = np.zeros((D, NC * VS), np.float32)
    wlmp[:, :V] = wlme

    masks = []
    for core in range(NC):
        cc = core % 4
        qpos = cc * CH + np.arange(CH)
        m = np.empty((128, 4, 2, CH), np.float32)
        for kc in range(4):
            for kt in range(2):
                kpos = kc * CH + kt * 128 + np.arange(128)
                m[:, kc, kt, :] = (kpos[:, None] <= qpos[None, :]).astype(np.float32)
        masks.append(m.astype(bf))

    shared = dict(wq=_to_pmaj(wqe.astype(bf)), wk=_to_pmaj(wke.astype(bf)),
                  wv=_to_pmaj(wve.astype(bf)), wo=_to_pmaj(Wo.astype(bf)),
                  w1=_to_pmaj(w1e.astype(bf)), w2=_to_pmaj(W2.astype(bf)),
                  bqk=bqk, b1t=b1t, bo2=np.ascontiguousarray(bo2))
    in_maps = []
    for core in range(NC):
        m = dict(shared)
        m["x0"] = np.ascontiguousarray(x0[core])
        m["msk"] = masks[core]
        m["wlm"] = _to_pmaj(wlmp[:, core * VS:(core + 1) * VS].astype(bf))
        in_maps.append(m)
    return in_maps, blme


def _run(inputs, trace=False):
    in_maps, blme = _prep_inputs(inputs)
    with_bias = bool(np.any(in_maps[0]["bo2"]))
    nc = _get_program(with_bias=with_bias)
    res = bass_utils.run_bass_kernel_spmd(nc, in_maps, core_ids=list(range(NC)),
                                          trace=trace)
    lg = np.concatenate(
        [np.asarray(res.results[c]["logits"]).astype(np.float32)
         .reshape(BT, NTP)[:, :VS] for c in range(NC)], axis=1)
    out = lg[:, :V]
    if np.any(blme):
        out = out + blme[None, :]
    return out.reshape(B, T, V).astype(np.float32), res


def kernel(**inputs) -> np.ndarray:
    out, _ = _run(inputs, trace=False)
    return out


# revision 24
# speedup vs baseline: 1.1651x; 1.0583x over previous
"""Trainium2 Bass kernel for a 6-layer GPT (D=512, H=8, T=1024, B=2, V=50257).

Strategy (8 NeuronCores), v3:
- Token-shard the transformer body: core c owns 256 tokens (cores 0-3 =
  batch 0 chunks 0-3, cores 4-7 = batch 1 chunks 0-3).
- All matmul operands bf16 (PSUM accumulates fp32); residual/LN fp32.
- Host pre-rearranges every weight into partition-major layout so DMA
  descriptors are >=4KB contiguous lines; DMAs are spread over the three
  trigger queues (sync / scalar / gpsimd).
- Per layer: LN1 -> QKV -> AllGather K,V (bf16) across the batch group
  (next layer's weights prefetched at layer start) -> causal attention,
  per-head PSUM accumulation, masks multiplied on the vector engine,
  per-head pipelined softmax normalization -> Wo + residual -> LN2 ->
  MLP -> residual.
- Final LN -> AllGather hidden (8-core, Shared/RDH) overlapped with
  discarded warmup matmuls on the local tokens -> vocab-sharded LM head
  with Wlm resident in SBUF (loaded in 4 slices during layers 0-2);
  logits accumulate 13 vocab tiles in SBUF and go out as one 13KB-per-
  partition DMA per 128-token group, alternating queues.
- Host folds LN gamma/beta and the 1/sqrt(HS) score scale into the
  weights; embedding gather happens host-side (tiny).
"""

import numpy as np
import ml_dtypes

import concourse.bass as bass
import concourse.tile as tile
from concourse import bacc, mybir
from concourse import bass_utils
from concourse.bass import ds, ts
from concourse.masks import make_identity

FP = mybir.dt.float32
BF = mybir.dt.bfloat16
AF = mybir.ActivationFunctionType
OP = mybir.AluOpType

V, D, T, L, H, HS, B = 50257, 512, 1024, 6, 8, 64, 2
FF = 4 * D
EPS = 1e-5
NC = 8          # cores
CH = 256        # tokens per core
VS = 6284       # padded vocab shard per core; 8*VS = 50272 >= V
KD = D // 128   # 4 k-tiles over D
MD = FF // 128  # 16 m-tiles over FF
BT = B * T
NT = (VS + 511) // 512   # 13 vocab tiles per core
NTP = NT * 512           # padded vocab shard (6656)


def build_program(with_bias=True, layers=L):
    nc = bacc.Bacc("TRN2", target_bir_lowering=False, debug=False, num_devices=NC)

    # ---- I/O (weights host-side pre-rearranged to partition-major) ----
    x0 = nc.dram_tensor("x0", [128, 2, D], FP, kind="ExternalInput").ap()
    wq = nc.dram_tensor("wq", [L, 128, KD, D], BF, kind="ExternalInput").ap()
    wk = nc.dram_tensor("wk", [L, 128, KD, D], BF, kind="ExternalInput").ap()
    wv = nc.dram_tensor("wv", [L, 128, KD, D], BF, kind="ExternalInput").ap()
    wo = nc.dram_tensor("wo", [L, 128, KD, D], BF, kind="ExternalInput").ap()
    w1 = nc.dram_tensor("w1", [L, 128, KD, FF], BF, kind="ExternalInput").ap()
    w2 = nc.dram_tensor("w2", [L, 128, MD, D], BF, kind="ExternalInput").ap()
    wlm = nc.dram_tensor("wlm", [128, KD, VS], BF, kind="ExternalInput").ap()
    bqk = nc.dram_tensor("bqk", [128, L, 2, KD], FP, kind="ExternalInput").ap()
    b1t = nc.dram_tensor("b1t", [128, L, MD], FP, kind="ExternalInput").ap()
    bo2 = nc.dram_tensor("bo2", [L, 2, D], FP, kind="ExternalInput").ap()
    # causal 0/1 mask per core: [p, kchunk, ktile, 256 queries] bf16
    msk = nc.dram_tensor("msk", [128, 4, 2, CH], BF, kind="ExternalInput").ap()
    # logits row (tg, p) = token tg*128+p of the gathered order
    logits = nc.dram_tensor("logits", [BT // 128, 128, NTP], BF,
                            kind="ExternalOutput").ap()

    KV_K = 128 * KD * CH            # kT flat elements per core
    KV_V = 128 * 2 * H * 65         # v_aug flat elements per core
    KV = KV_K + KV_V
    XF = 128 * KD * CH              # xfT flat elements

    from contextlib import ExitStack
    with ExitStack() as stk:
        tc = stk.enter_context(tile.TileContext(nc))
        ec = stk.enter_context
        consts = ec(tc.tile_pool(name="consts", bufs=1))
        wlmp = ec(tc.tile_pool(name="wlmp", bufs=1))
        xpool = ec(tc.tile_pool(name="xpool", bufs=1))
        hpool = ec(tc.tile_pool(name="hpool", bufs=1))
        t4 = ec(tc.tile_pool(name="t4", bufs=4))          # [128,KD,CH] transposed acts
        wqkvop = ec(tc.tile_pool(name="wqkvo", bufs=2))   # [128,KD,4,512]
        w1pool = ec(tc.tile_pool(name="w1k", bufs=3))     # [128,KD,FF//2] halves
        w2pool = ec(tc.tile_pool(name="w2k", bufs=3))     # [128,MD//2,D] halves
        kvall = ec(tc.tile_pool(name="kvall", bufs=1))
        vaugp = ec(tc.tile_pool(name="vaug", bufs=1))
        small = ec(tc.tile_pool(name="small", bufs=2))
        expp = ec(tc.tile_pool(name="exp", bufs=2))
        gtp = ec(tc.tile_pool(name="gt", bufs=2))
        attp = ec(tc.tile_pool(name="attp", bufs=1))
        rbcp = ec(tc.tile_pool(name="rbc", bufs=2))
        lgp = ec(tc.tile_pool(name="lg", bufs=2))
        bcp = ec(tc.tile_pool(name="bcast", bufs=2))
        xftp = ec(tc.tile_pool(name="xft", bufs=1))
        xfap = ec(tc.tile_pool(name="xfa", bufs=2))
        mmp = ec(tc.tile_pool(name="mm", bufs=2, space="PSUM"))
        avp = ec(tc.tile_pool(name="avp", bufs=2, space="PSUM"))
        spp = ec(tc.tile_pool(name="sp", bufs=2, space="PSUM"))
        mop = ec(tc.tile_pool(name="mo", bufs=2, space="PSUM"))
        dram = ec(tc.tile_pool(name="dram", bufs=2, space="DRAM"))

        # ---- consts + first activations on the sync queue ----
        xt = xpool.tile([128, 2, D], FP, tag="xt")
        nc.sync.dma_start(xt[:], x0)
        ident = consts.tile([128, 128], BF)
        make_identity(nc, ident)
        epst = consts.tile([128, 1], FP)
        nc.vector.memset(epst, EPS)
        msk_sb = consts.tile([128, 4, 2, CH], BF)
        nc.sync.dma_start(msk_sb[:], msk)
        bqk_sb = consts.tile([128, L, 2, KD], FP)
        b1_sb = consts.tile([128, L, MD], FP)
        if with_bias:
            nc.sync.dma_start(bqk_sb[:], bqk)
            nc.sync.dma_start(b1_sb[:], b1t)

        # ---- layer-0 weights (scalar + gpsimd queues), then resident Wlm --
        def load_wqkvo(l):
            w = wqkvop.tile([128, KD, 4, 512], BF, tag="wqkvo", name=f"wqkvo{l}")
            for i, src in enumerate((wq, wk, wv, wo)):
                eng = nc.scalar if i < 2 else nc.gpsimd
                eng.dma_start(w[:, :, i, :], src[l])
            return w

        def load_w1h(l, i):
            w = w1pool.tile([128, KD, FF // 2], BF, tag="w1k",
                            name=f"w1k{l}_{i}")
            nc.scalar.dma_start(w[:], w1[l, :, :, ds(i * (FF // 2), FF // 2)])
            return w

        def load_w2h(l, i):
            w = w2pool.tile([128, MD // 2, D], BF, tag="w2k",
                            name=f"w2k{l}_{i}")
            nc.gpsimd.dma_start(w[:], w2[l, :, ds(i * (MD // 2), MD // 2), :])
            return w

        cur_wqkvo = load_wqkvo(0)
        cur_w1 = [load_w1h(0, 0), load_w1h(0, 1)]
        cur_w2 = [load_w2h(0, 0), load_w2h(0, 1)]
        wlm_sb = wlmp.tile([128, KD, VS], BF)
        nc.gpsimd.dma_start(wlm_sb[:, 0, :], wlm[:, 0, :])

        def layernorm(src):
            """src [128,2,D] fp32 -> normalized [128,2,D] bf16."""
            out = hpool.tile([128, 2, D], BF, tag="h")
            for tt in range(2):
                st = small.tile([128, 6], FP, tag="bnst")
                nc.vector.bn_stats(st[:], src[:, tt, :])
                mv = small.tile([128, 2], FP, tag="bnmv")
                nc.vector.bn_aggr(mv[:], st[:])
                nc.scalar.activation(mv[:, 1:2], mv[:, 1:2], AF.Sqrt,
                                     bias=epst[:, 0:1])
                nc.vector.reciprocal(mv[:, 1:2], mv[:, 1:2])
                nc.vector.tensor_scalar(
                    out=out[:, tt, :], in0=src[:, tt, :],
                    scalar1=mv[:, 0:1], scalar2=mv[:, 1:2],
                    op0=OP.subtract, op1=OP.mult)
            return out

        def transpose2(src, pool=t4):
            """src [128,2,D] bf16 (tokens, dims) -> [128,KD,CH] (dims, toks)."""
            out = pool.tile([128, KD, CH], BF, tag="t4" if pool is t4 else "xft")
            for d in range(KD):
                for tt in range(2):
                    ps = mmp.tile([128, 1024], BF, tag="mm")
                    nc.tensor.transpose(ps[:, 0:128], src[:, tt, ds(d * 128, 128)],
                                        ident[:])
                    nc.vector.tensor_copy(out[:, d, ds(tt * 128, 128)],
                                          ps[:, 0:128])
            return out

        nxt_wqkvo = nxt_w1h = nxt_w2h = None
        for l in range(layers):
            # ---- prefetch next layer's weights + one Wlm slice ----
            if l + 1 < layers:
                nxt_wqkvo = load_wqkvo(l + 1)
                nxt_w1h = load_w1h(l + 1, 0)
                nxt_w2h = load_w2h(l + 1, 0)
            if l < KD - 1:
                nc.gpsimd.dma_start(wlm_sb[:, l + 1, :], wlm[:, l + 1, :])

            # ---- LN1 + transpose ----
            h = layernorm(xt)
            hT = transpose2(h)

            # ---- k/v first so the AllGather starts ASAP ----
            kT = t4.tile([128, KD, CH], BF, tag="t4")
            for d in range(KD):
                ps = mmp.tile([128, 512], FP, tag="mm")
                for k in range(KD):
                    nc.tensor.matmul(ps[:, :CH], cur_wqkvo[:, k, 1, ds(d * 128, 128)],
                                     hT[:, k, :], start=(k == 0),
                                     stop=(k == KD - 1))
                if with_bias:
                    nc.vector.tensor_scalar_add(kT[:, d, :], ps[:, :CH],
                                                bqk_sb[:, l, 1, d:d + 1])
                else:
                    nc.vector.tensor_copy(kT[:, d, :], ps[:, :CH])
            vaug = vaugp.tile([128, 2, H, 65], BF)
            nc.vector.memset(vaug[:, :, :, 64:65], 1.0)
            for tt in range(2):
                ps = mmp.tile([128, 512], FP, tag="mm")
                for k in range(KD):
                    nc.tensor.matmul(ps[:, :D], hT[:, k, ds(tt * 128, 128)],
                                     cur_wqkvo[:, k, 2, :], start=(k == 0),
                                     stop=(k == KD - 1))
                nc.vector.tensor_copy(
                    vaug[:, tt, :, 0:64],
                    ps[:].rearrange("p (h e) -> p h e", h=H))

            # ---- AllGather K,V across batch group (bf16, split queues) ----
            kv_in = dram.tile([KV], BF, tag="kvin")
            nc.sync.dma_start(
                kv_in[0:KV_K].rearrange("(p a b) -> p a b", p=128, a=KD), kT[:])
            nc.scalar.dma_start(
                kv_in[KV_K:KV].rearrange("(p a h e) -> p a h e", p=128, a=2, h=H),
                vaug[:])
            kv_out = dram.tile([4, KV], BF, tag="kvout")
            nc.gpsimd.collective_compute(
                "AllGather", OP.bypass,
                replica_groups=[[0, 1, 2, 3], [4, 5, 6, 7]],
                ins=[kv_in[:].opt()], outs=[kv_out[:].opt()])

            # ---- q projection overlaps the collective ----
            qT = t4.tile([128, KD, CH], BF, tag="t4")
            for d in range(KD):
                ps = mmp.tile([128, 512], FP, tag="mm")
                for k in range(KD):
                    nc.tensor.matmul(ps[:, :CH], cur_wqkvo[:, k, 0, ds(d * 128, 128)],
                                     hT[:, k, :], start=(k == 0),
                                     stop=(k == KD - 1))
                if with_bias:
                    nc.vector.tensor_scalar_add(qT[:, d, :], ps[:, :CH],
                                                bqk_sb[:, l, 0, d:d + 1])
                else:
                    nc.vector.tensor_copy(qT[:, d, :], ps[:, :CH])

            kTall = kvall.tile([128, KD, 4, CH], BF, tag="ktall")
            vall = kvall.tile([128, 4, 2, H, 65], BF, tag="vall")
            for c in range(4):
                nc.sync.dma_start(
                    kTall[:, :, c, :],
                    kv_out[c, 0:KV_K].rearrange("(p a b) -> p a b", p=128, a=KD))
                nc.scalar.dma_start(
                    vall[:, c, :, :, :],
                    kv_out[c, KV_K:KV].rearrange("(p a h e) -> p a h e",
                                                 p=128, a=2, h=H))

            # ---- attention: per-head PSUM accumulation over the 4 chunks.
            # attT is assembled with RAW (unnormalized) values; denominators
            # collect in dram, then one compact reciprocal + a broadcast DMA
            # into transposed layout + a single in-place multiply normalize.
            attT = t4.tile([128, KD, CH], BF, tag="t4")
            # araw rows 0-63 = numerators; row 64 = denominators
            araw = attp.tile([128, H, CH], BF, tag="araw", name=f"araw{l}")
            rdram = dram.tile([H * CH], BF, tag="rdram")
            rdram2 = dram.tile([H * CH], BF, tag="rdram2")
            for hh in range(H):
                pb = (hh % 2) * 64
                dt_ = hh // 2
                avps = avp.tile([65, CH], FP, tag="av")
                for c in range(4):
                    sps = spp.tile([128, 2, CH], FP, tag="sp")
                    for kt in range(2):
                        nc.tensor.matmul(
                            sps[:, kt, :],
                            kTall[pb:pb + 64, dt_, c, ds(kt * 128, 128)],
                            qT[pb:pb + 64, dt_, :],
                            start=True, stop=True)
                    ex = expp.tile([128, 2, CH], BF, tag="exp")
                    nc.scalar.activation(ex[:], sps[:], AF.Exp)
                    # alternate mask engine so neither becomes the straggler
                    if c % 2 == 0:
                        nc.vector.tensor_tensor(ex[:], ex[:], msk_sb[:, c, :, :],
                                                OP.mult)
                    else:
                        nc.gpsimd.tensor_tensor(ex[:], ex[:], msk_sb[:, c, :, :],
                                                OP.mult)
                    for kt in range(2):
                        nc.tensor.matmul(
                            avps[:], vall[:, c, kt, hh, :], ex[:, kt, :],
                            start=(c == 0 and kt == 0),
                            stop=(c == 3 and kt == 1))
                # eagerly drain PSUM so later heads are never blocked
                if pb == 0:
                    nc.vector.tensor_copy(attT[0:64, dt_, :], avps[0:64, :])
                    nc.vector.tensor_copy(araw[64:65, hh, :], avps[64:65, :])
                else:
                    nc.vector.tensor_copy(araw[0:65, hh, :], avps[0:65, :])
                    nc.sync.dma_start(attT[64:128, dt_, :], araw[0:64, hh, :])
                nc.sync.dma_start(
                    rdram[:].rearrange("(a b) -> a b", a=H)[hh:hh + 1, :],
                    araw[64:65, hh, :])
            rsb = small.tile([128, H * CH // 128], BF, tag="rsb")
            nc.sync.dma_start(rsb[:], rdram[:].rearrange("(p f) -> p f", p=128))
            with nc.allow_low_precision(reason="softmax denom recip bf16"):
                nc.vector.reciprocal(rsb[:], rsb[:])
            nc.sync.dma_start(rdram2[:].rearrange("(p f) -> p f", p=128), rsb[:])
            rbcT = rbcp.tile([128, KD, CH], BF, tag="rbcT", name=f"rbcT{l}")
            for pa in range(2):
                nc.sync.dma_start(
                    rbcT[pa * 64:(pa + 1) * 64, :, :],
                    bass.AP(tensor=rdram2.tensor,
                            offset=rdram2.offset + pa * CH,
                            ap=[[0, 64], [2 * CH, KD], [1, CH]]))
            nc.vector.tensor_tensor(attT[:], attT[:], rbcT[:], OP.mult)

            # ---- Wo + bias + residual ----
            if with_bias:
                bo_b = bcp.tile([128, D], FP, tag="bc")
                bo_src = bo2[l, 0]
                nc.sync.dma_start(bo_b[:], bass.AP(
                    tensor=bo_src.tensor, offset=bo_src.offset,
                    ap=[[0, 128]] + list(bo_src.ap)))
            for tt in range(2):
                ps = mmp.tile([128, 512], FP, tag="mm")
                for k in range(KD):
                    nc.tensor.matmul(ps[:, :D], attT[:, k, ds(tt * 128, 128)],
                                     cur_wqkvo[:, k, 3, :], start=(k == 0),
                                     stop=(k == KD - 1))
                if with_bias:
                    nc.vector.tensor_tensor(ps[:, :D], ps[:, :D], bo_b[:],
                                            OP.add)
                nc.vector.tensor_tensor(xt[:, tt, :], xt[:, tt, :], ps[:, :D],
                                        OP.add)

            # ---- LN2 + transpose ----
            h2 = layernorm(xt)
            h2T = transpose2(h2)

            # ---- MLP ----
            if with_bias:
                b2_b = bcp.tile([128, D], FP, tag="bc")
                b2_src = bo2[l, 1]
                nc.sync.dma_start(b2_b[:], bass.AP(
                    tensor=b2_src.tensor, offset=b2_src.offset,
                    ap=[[0, 128]] + list(b2_src.ap)))
            x2ps = [mop.tile([128, D], FP, tag="mo", name=f"mo{l}_{kk}")
                    for kk in range(2)]
            for mp in range(MD // 2):
                gps = mmp.tile([128, 2, CH], FP, tag="mm")
                for mi in range(2):
                    m = mp * 2 + mi
                    w1h = cur_w1[m // (MD // 2)]
                    mm = m % (MD // 2)
                    for k in range(KD):
                        nc.tensor.matmul(gps[:, mi, :],
                                         w1h[:, k, ds(mm * 128, 128)],
                                         h2T[:, k, :], start=(k == 0),
                                         stop=(k == KD - 1))
                gt = gtp.tile([128, 2, CH], BF, tag="gt")
                if with_bias:
                    for mi in range(2):
                        m = mp * 2 + mi
                        nc.scalar.activation(gt[:, mi, :], gps[:, mi, :], AF.Gelu,
                                             bias=b1_sb[:, l, m:m + 1])
                else:
                    nc.scalar.activation(gt[:], gps[:], AF.Gelu)
                for mi in range(2):
                    m = mp * 2 + mi
                    w2h = cur_w2[m // (MD // 2)]
                    for tt in range(2):
                        nc.tensor.matmul(x2ps[tt][:], gt[:, mi, ds(tt * 128, 128)],
                                         w2h[:, m % (MD // 2), :],
                                         start=(m == 0), stop=(m == MD - 1))
                if mp == 3 and l + 1 < layers:
                    # the current layer's first halves had their last read at
                    # m=7 (this iteration) - their ring slots are now free
                    nxt_w1 = [nxt_w1h, load_w1h(l + 1, 1)]
                    nxt_w2 = [nxt_w2h, load_w2h(l + 1, 1)]
            for tt in range(2):
                if with_bias:
                    nc.vector.tensor_tensor(x2ps[tt][:], x2ps[tt][:], b2_b[:],
                                            OP.add)
                nc.vector.tensor_tensor(xt[:, tt, :], xt[:, tt, :], x2ps[tt][:],
                                        OP.add)
            if l + 1 < layers:
                cur_wqkvo, cur_w1, cur_w2 = nxt_wqkvo, nxt_w1, nxt_w2

        # ---- final LN; AllGather hidden state across all 8 (Shared/RDH) ----
        xf = layernorm(xt)
        xfT = transpose2(xf, pool=xftp)
        xf_in = dram.tile([XF], BF, tag="xfin")
        nc.sync.dma_start(
            xf_in[:].rearrange("(p a b) -> p a b", p=128, a=KD), xfT[:])
        xf_out = dram.tile([NC, XF], BF, tag="xfout", addr_space="Shared")
        nc.gpsimd.collective_compute(
            "AllGather", OP.bypass,
            replica_groups=[list(range(NC))],
            ins=[xf_in[:].opt()], outs=[xf_out[:].opt()])

        hp_pools = ((mmp, "mm"), (mop, "mo"), (spp, "sp"))
        dma_engs = (nc.sync, nc.scalar, nc.gpsimd)
        gi = 0
        LGB = [0, 5, 9, 13]  # logits go out in chunks of 5/4/4 vocab tiles

        def head_group(get_lhs, tg):
            """One 128-token group: all NT vocab tiles -> three chunked DMAs.
            tg None => warmup-only (compute and discard)."""
            nonlocal gi
            lgt = None
            cj = 0
            for n in range(NT):
                if tg is not None and n == LGB[cj]:
                    lgt = lgp.tile([128, 5, 512], BF, tag="lg",
                                   name=f"lg{tg}_{cj}")
                nsz = min(512, VS - n * 512)
                pool_i, ptag = hp_pools[gi % 3]
                ps = pool_i.tile([128, 512], FP, tag=ptag,
                                 name=f"hps{tg}_{n}")
                for k in range(KD):
                    nc.tensor.matmul(
                        ps[:, :nsz], get_lhs(k),
                        wlm_sb[:, k, ds(n * 512, nsz)],
                        start=(k == 0), stop=(k == KD - 1))
                gi += 1
                if tg is None:
                    continue
                nb = n - LGB[cj]
                if gi % 2 == 0:
                    nc.vector.tensor_copy(lgt[:, nb, :nsz], ps[:, :nsz])
                else:
                    nc.scalar.activation(lgt[:, nb, :nsz], ps[:, :nsz], AF.Copy)
                if n == LGB[cj + 1] - 1:
                    nw = LGB[cj + 1] - LGB[cj]
                    dma_engs[(tg + cj) % 3].dma_start(
                        logits[tg, :, ds(LGB[cj] * 512, nw * 512)],
                        lgt[:, 0:nw, :])
                    cj += 1

        # pass 1: warmup on local tokens while the AllGather flies (discard)
        for mt in range(2):
            head_group(lambda k, mt=mt: xfT[:, k, ds(mt * 128, 128)], None)

        # read the gathered slots back (alternate queues), then pass 2
        xfall = []
        for cg in range(2):
            xa = xfap.tile([128, KD, 4, CH], BF, tag="xfa", name=f"xfa{cg}")
            for c in range(4):
                dma_engs[(cg * 4 + c) % 3].dma_start(
                    xa[:, :, c, :],
                    xf_out[cg * 4 + c, :].rearrange("(p a b) -> p a b",
                                                    p=128, a=KD))
            xfall.append(xa)
        for c in range(NC):
            for mt in range(2):
                head_group(
                    lambda k, c=c, mt=mt:
                        xfall[c // 4][:, k, c % 4, ds(mt * 128, 128)],
                    c * 2 + mt)

    nc.compile()
    return nc


_CACHE = {}


def _get_program(with_bias=True):
    key = ("nc", with_bias)
    if key not in _CACHE:
        _CACHE[key] = build_program(with_bias=with_bias)
    return _CACHE[key]


def _to_pmaj(w):
    """[.., D_in, N] -> [.., 128, D_in//128, N] partition-major."""
    shp = w.shape
    kd = shp[-2] // 128
    return np.ascontiguousarray(
        w.reshape(*shp[:-2], kd, 128, shp[-1]).swapaxes(-3, -2))


def _prep_inputs(inputs):
    f = lambda k: np.asarray(inputs[k], np.float32)
    bf = ml_dtypes.bfloat16
    idx = np.asarray(inputs["idx"]).astype(np.int64)
    tok_emb, pos_emb = f("tok_emb"), f("pos_emb")
    x0 = tok_emb[idx] + pos_emb[None, :T]          # [B, T, D]
    x0 = x0.reshape(NC, 2, 128, D).transpose(0, 2, 1, 3).copy()  # [NC,128,2,D]

    ln1_g, ln1_b = f("ln1_g"), f("ln1_b")
    ln2_g, ln2_b = f("ln2_g"), f("ln2_b")
    Wq, bq = f("Wq"), f("bq")
    Wk, bk = f("Wk"), f("bk")
    Wv, bv = f("Wv"), f("bv")
    Wo, bo = f("Wo"), f("bo")
    W1, b1 = f("W1"), f("b1")
    W2, b2 = f("W2"), f("b2")
    lnf_g, lnf_b = f("lnf_g"), f("lnf_b")
    Wlm, blm = f("Wlm"), f("blm")

    sc = 1.0 / np.sqrt(HS)
    wqe = ln1_g[:, :, None] * Wq * sc
    bqe = (np.einsum("ld,ldm->lm", ln1_b, Wq) + bq) * sc
    wke = ln1_g[:, :, None] * Wk
    bke = np.einsum("ld,ldm->lm", ln1_b, Wk) + bk
    wve = ln1_g[:, :, None] * Wv
    bve = np.einsum("ld,ldm->lm", ln1_b, Wv) + bv
    boe = np.einsum("lm,lmd->ld", bve, Wo) + bo
    w1e = ln2_g[:, :, None] * W1
    b1e = np.einsum("ld,ldf->lf", ln2_b, W1) + b1
    wlme = lnf_g[:, None] * Wlm
    blme = lnf_b @ Wlm + blm

    bqk = np.stack([bqe, bke], axis=1)             # [L, 2, D]
    bqk = bqk.reshape(L, 2, KD, 128).transpose(3, 0, 1, 2).copy()
    b1t = b1e.reshape(L, MD, 128).transpose(2, 0, 1).copy()
    bo2 = np.stack([boe, b2], axis=1)              # [L, 2, D]

    wlmp = np.zeros((D, NC * VS), np.float32)
    wlmp[:, :V] = wlme

    masks = []
    for core in range(NC):
        cc = core % 4
        qpos = cc * CH + np.arange(CH)
        m = np.empty((128, 4, 2, CH), np.float32)
        for kc in range(4):
            for kt in range(2):
                kpos = kc * CH + kt * 128 + np.arange(128)
                m[:, kc, kt, :] = (kpos[:, None] <= qpos[None, :]).astype(np.float32)
        masks.append(m.astype(bf))

    shared = dict(wq=_to_pmaj(wqe.astype(bf)), wk=_to_pmaj(wke.astype(bf)),
                  wv=_to_pmaj(wve.astype(bf)), wo=_to_pmaj(Wo.astype(bf)),
                  w1=_to_pmaj(w1e.astype(bf)), w2=_to_pmaj(W2.astype(bf)),
                  bqk=bqk, b1t=b1t, bo2=np.ascontiguousarray(bo2))
    in_maps = []
    for core in range(NC):
        m = dict(shared)
        m["x0"] = np.ascontiguousarray(x0[core])
        m["msk"] = masks[core]
        m["wlm"] = _to_pmaj(wlmp[:, core * VS:(core + 1) * VS].astype(bf))
        in_maps.append(m)
    return in_maps, blme


def _run(inputs, trace=False):
    in_maps, blme = _prep_inputs(inputs)
    with_bias = bool(np.any(in_maps[0]["bo2"]))
    nc = _get_program(with_bias=with_bias)
    res = bass_utils.run_bass_kernel_spmd(nc, in_maps, core_ids=list(range(NC)),
                                          trace=trace)
    lg = np.concatenate(
        [np.asarray(res.results[c]["logits"]).astype(np.float32)
         .reshape(BT, NTP)[:, :VS] for c in range(NC)], axis=1)
    out = lg[:, :V]
    if np.any(blme):
        out = out + blme[None, :]
    return out.reshape(B, T, V).astype(np.float32), res


def kernel(**inputs) -> np.ndarray:
    out, _ = _run(inputs, trace=False)
    return out
